# revision 40
# baseline (speedup 1.0000x reference)
"""Trainium2 Bass kernel for CombinedRankingLoss (BCE + pairwise margin ranking).

Full inputs: logits/labels/weights [64, 1024, 1] f32. Output: scalar f32.

Data-parallel over batch: 8 cores x 8 batches. The pairwise term
    T_b = sum_{i in pos} sum_{j in neg} relu((v_j + M) - v_i)
is computed per batch with the candidates PERMUTED on host (the loss is
invariant to per-batch candidate order):
  - a side (i): pos-compacted, padded to KA=640 with +BIG  (sent negated)
  - b side (j): neg-compacted (v+M), padded to KB=640 with -BIG, and sent
    as a bf16 hi/lo pair whose sum reconstructs f32 to ~1e-4
Masked pads contribute exactly 0 through the relu.

Per batch on device:
  - PE: one [16,128] bf16 selector matmul broadcasts (hi_b + lo_b) rows to a
    [128, KB] f32 PSUM tile.
  - 5 chunk ops (128 pos candidates each) fused compare+row-reduce, one
    instruction each, split across ScalarE and VectorE (both read PSUM):
      ScalarE: activation(Relu, bias=-a_col, accum_out)  -> sum_j relu(b_j-a_i)
      VectorE: tensor_scalar(op0=max a_col, op1=add-as-REDUCER, accum_out)
               -> sum_j max(b_j, a_i); host subtracts KB*a_i (exact identity
               sum_j max(b_j,a)-KB*a = sum_j relu(b_j-a))
  - BCE partials on flat [128,64] tiles (softplus = ln(1+exp), one ACT table
    set shared with Relu).
  - ones-matmul reduces accumulator partitions -> [1,48] -> DMA out.
Host: per-batch counts/means/valid handling + final scalar (f64).
"""
import sys
import numpy as np

sys.path.insert(0, "/opt/trn_rl_repo")

B, N = 64, 1024
N_CORES = 8
BLOC = B // N_CORES          # batches per core
KA = 640                     # padded pos-count (i side); 5 chunks of 128
KB = 576                     # padded neg-count (j side, free dim); overflow
                             # falls back to exact host recompute per batch
NCH = KA // 128
MARGIN = 0.5
BIG = 16.0                   # mask sentinel; dominates |v|+margin (|v|<~5.5)
NACC = NCH * BLOC            # 40 accumulator columns
NOUT = 48

_CACHE = {}


def _patch_bass(bass):
    """Split multi-wait instructions (old walrus TPB_CTRL takes 1 wait)."""
    import json as _json
    if getattr(bass.Bass, "_wait_split_patched", False):
        return
    _orig = bass.Bass.to_json_bytes

    def _split(bir, limit=1):
        m = _json.loads(bir)
        for fn in m["functions"]:
            for bb in fn["blocks"]:
                out = []
                for i in bb.get("instructions", []):
                    si = i.get("sync_info") or {}
                    ow = si.get("on_wait") or []
                    if len(ow) > limit:
                        extra, keep = ow[:-limit], ow[-limit:]
                        for k, w in enumerate(extra):
                            out.append({
                                "debug": i.get("debug"), "engine": i["engine"],
                                "ins": [], "outs": [],
                                "name": i["name"] + f"_ws{k}",
                                "opcode": "NoOp",
                                "sync_info": {"on_wait": [w]},
                            })
                        si = dict(si)
                        si["on_wait"] = keep
                        i = dict(i)
                        i["sync_info"] = si
                    out.append(i)
                bb["instructions"] = out
        return _json.dumps(m).encode()

    bass.Bass.to_json_bytes = lambda self: _split(_orig(self))
    bass.Bass._wait_split_patched = True


def _engine_for(k):
    # 20:20 split (measured contended rates ~equal): even batches DVE gets
    # chunks {0,1,4}, odd batches {0,1}
    b, c = divmod(k, NCH)
    if b % 2 == 0:
        return "dve" if c in (0, 1, 4) else "act"
    return "dve" if c in (0, 1) else "act"


def _build(bass, tile, mybir):
    f32 = mybir.dt.float32
    bf16 = mybir.dt.bfloat16
    Alu = mybir.AluOpType
    Act = mybir.ActivationFunctionType

    nc = bass.Bass()
    na_d = nc.declare_dram_parameter("na", [BLOC, KA], f32, isOutput=False)
    b2_d = nc.declare_dram_parameter("b2", [16, KB], bf16, isOutput=False)
    v_d = nc.declare_dram_parameter("v", [128, 64], f32, isOutput=False)
    y_d = nc.declare_dram_parameter("y", [128, 64], f32, isOutput=False)
    w_d = nc.declare_dram_parameter("w", [128, 64], f32, isOutput=False)
    sel_d = nc.declare_dram_parameter("sel", [16, N], bf16, isOutput=False)
    id8_d = nc.declare_dram_parameter("id8", [8, 8], f32, isOutput=False)
    outd_d = nc.declare_dram_parameter("outd", [128, NOUT], f32, isOutput=True)
    outa_d = nc.declare_dram_parameter("outa", [128, NOUT], f32, isOutput=True)

    with tile.TileContext(nc) as tc:
        with (
            tc.tile_pool(name="const", bufs=1) as const,
            tc.tile_pool(name="work", bufs=2) as work,
            tc.tile_pool(name="psum", bufs=3, space="PSUM") as psum,
            tc.tile_pool(name="psum1", bufs=1, space="PSUM") as psum1,
        ):
            # ---------- loads (matmul/transpose inputs first) ----------
            b2 = const.tile([16, KB], bf16)
            nc.sync.dma_start(out=b2[:], in_=b2_d[:])
            sel16 = const.tile([16, N], bf16)
            nc.sync.dma_start(out=sel16[:], in_=sel_d[:])
            na_rows = const.tile([BLOC, KA], f32)
            nc.sync.dma_start(out=na_rows[:], in_=na_d[:])
            ident8 = const.tile([8, 8], f32)
            nc.sync.dma_start(out=ident8[:], in_=id8_d[:])
            v_flat = const.tile([128, 64], f32)
            y_flat = const.tile([128, 64], f32)
            w_flat = const.tile([128, 64], f32)
            nc.sync.dma_start(out=v_flat[:], in_=v_d[:])
            nc.sync.dma_start(out=y_flat[:], in_=y_d[:])
            nc.sync.dma_start(out=w_flat[:], in_=w_d[:])

            # accumulators
            dve_acc = const.tile([128, NOUT], f32)
            act_acc = psum1.tile([128, NOUT], f32)   # ScE is closer to PSUM
            nc.vector.memset(dve_acc[:], 0.0)
            nc.vector.memset(act_acc[:], 0.0)

            # ---------- a-columns via PE transpose ----------
            tp = psum1.tile([128, NCH * 8], f32)
            for c in range(NCH):
                nc.tensor.transpose(tp[:, c * 8:(c + 1) * 8],
                                    na_rows[:, c * 128:(c + 1) * 128], ident8[:])
            na_cols = const.tile([128, NCH * 8], f32)   # -a  (ACT bias)
            nc.vector.tensor_copy(out=na_cols[:], in_=tp[:])
            a_cols = const.tile([128, NCH * 8], f32)    # +a  (DVE max operand)
            nc.vector.tensor_scalar(out=a_cols[:], in0=tp[:], scalar1=-1.0,
                                    scalar2=None, op0=Alu.mult)

            # ---------- BCE (flat [128,64]) ----------
            sp = work.tile([128, 64], f32)
            nc.scalar.activation(out=sp[:], in_=v_flat[:], func=Act.Exp)
            nc.vector.tensor_scalar(out=sp[:], in0=sp[:], scalar1=1.0,
                                    scalar2=None, op0=Alu.add)
            nc.scalar.activation(out=sp[:], in_=sp[:], func=Act.Ln)
            xy = work.tile([128, 64], f32)
            nc.vector.tensor_tensor(out=xy[:], in0=v_flat[:], in1=y_flat[:], op=Alu.mult)
            nc.vector.tensor_tensor(out=xy[:], in0=sp[:], in1=xy[:], op=Alu.subtract)
            bce_scr = work.tile([128, 64], f32)
            nc.vector.scalar_tensor_tensor(
                out=bce_scr[:], in0=xy[:], scalar=1.0, op0=Alu.mult,
                op1=Alu.mult, in1=w_flat[:], accum_out=dve_acc[:, NACC:NACC + 1])

            # ---------- main pairwise loop (software-pipelined emission) ----
            def emit_mm(b):
                bcp = psum.tile([128, KB], f32, tag="bcp")    # pool pads to banks
                lhsT = sel16[:, b * 128:(b + 1) * 128]
                nc.tensor.matmul(bcp[:, 0:512], lhsT, b2[:, 0:512],
                                 start=True, stop=True)
                nc.tensor.matmul(bcp[:, 512:KB], lhsT, b2[:, 512:KB],
                                 start=True, stop=True)
                return bcp

            bcp_q = [emit_mm(b) for b in range(3)]
            for b in range(BLOC):
                bcp = bcp_q[b % 3]
                # DVE chunks issued first, then ACT chunks
                order = sorted(range(NCH), key=lambda c: _engine_for(b * NCH + c) != "dve")
                for c in order:
                    col = b * NCH + c
                    cidx = c * 8 + b
                    if _engine_for(b * NCH + c) == "act":
                        scr_act = work.tile([128, KB], f32, tag="scr_act")
                        nc.scalar.activation(
                            out=scr_act[:], in_=bcp[:], func=Act.Relu,
                            bias=na_cols[:, cidx:cidx + 1], scale=1.0,
                            accum_out=act_acc[:, col:col + 1])
                    else:
                        scr_dve = work.tile([128, KB], f32, tag="scr_dve")
                        nc.vector.tensor_scalar(
                            out=scr_dve[:], in0=bcp[:],
                            scalar1=a_cols[:, cidx:cidx + 1], scalar2=None,
                            op0=Alu.max, op1=Alu.add,
                            accum_out=dve_acc[:, col:col + 1])
                if b + 3 < BLOC:
                    bcp_q[(b + 3) % 3] = emit_mm(b + 3)

            # ---------- results out (host does the partition reduction) ----
            nc.sync.dma_start(out=outd_d[:], in_=dve_acc[:])
            acta_sb = const.tile([128, NOUT], f32)
            nc.vector.tensor_copy(out=acta_sb[:], in_=act_acc[:])
            nc.sync.dma_start(out=outa_d[:], in_=acta_sb[:])

    return nc


def _get_nc():
    if "nc" not in _CACHE:
        import concourse.bass as bass
        import concourse.tile as tile
        from concourse import mybir
        _patch_bass(bass)
        _CACHE["nc"] = _build(bass, tile, mybir)
    return _CACHE["nc"]


def _prep_core(v, y):
    """Compact one core's batches: returns na [BLOC,KA] f32, b2 [16,KB] bf16,
    a_pad [BLOC,KA] f64 (for the DVE correction), overflow list."""
    import ml_dtypes
    na = np.full((BLOC, KA), -BIG, dtype=np.float32)
    b_pad = np.full((BLOC, KB), -BIG, dtype=np.float32)
    overflow = []
    for r in range(BLOC):
        pos = v[r][y[r] == 1.0]
        neg = v[r][y[r] == 0.0] + np.float32(MARGIN)
        if len(pos) > KA or len(neg) > KB:
            overflow.append(r)   # leave na row at -BIG => device contributes 0
            continue
        na[r, :len(pos)] = -pos
        b_pad[r, :len(neg)] = neg
    hi = b_pad.astype(ml_dtypes.bfloat16)
    lo = (b_pad - hi.astype(np.float32)).astype(ml_dtypes.bfloat16)
    b2 = np.concatenate([hi, lo], axis=0)           # [16, KB]
    b_eff = hi.astype(np.float64) + lo.astype(np.float64)
    return na, b2, -na.astype(np.float64), b_eff, overflow


def _host_consts():
    import ml_dtypes
    sel = np.zeros((16, N), dtype=np.float32)
    for b in range(BLOC):
        sel[b, b * 128:(b + 1) * 128] = 1.0
        sel[b + 8, b * 128:(b + 1) * 128] = 1.0
    id8 = np.eye(8, dtype=np.float32)
    return sel.astype(ml_dtypes.bfloat16), id8


def make_in_maps(v, y, w):
    sel, id8 = _host_consts()
    in_maps, a_pads, overflows = [], [], []
    for c in range(N_CORES):
        sl = slice(c * BLOC, (c + 1) * BLOC)
        na, b2, a_pad, b_eff, ovf = _prep_core(v[sl], y[sl])
        a_pads.append(a_pad)
        overflows.append(ovf)
        in_maps.append({
            "na": na, "b2": b2,
            "v": v[sl].reshape(128, 64),
            "y": y[sl].reshape(128, 64),
            "w": w[sl].reshape(128, 64),
            "sel": sel, "id8": id8,
        })
    return in_maps, a_pads, overflows


def kernel(logits, labels, weights):
    from concourse.bass_utils import run_bass_kernel_spmd

    nc = _get_nc()
    v = np.ascontiguousarray(logits.reshape(B, N), dtype=np.float32)
    y = np.ascontiguousarray(labels.reshape(B, N), dtype=np.float32)
    w = np.ascontiguousarray(weights.reshape(B, N), dtype=np.float32)

    in_maps, a_pads, overflows = make_in_maps(v, y, w)
    res = run_bass_kernel_spmd(nc, in_maps, list(range(N_CORES)))

    # ---------- host combine ----------
    dve_col = np.array([_engine_for(k) == "dve" for k in range(NACC)])
    bce_sum = 0.0
    pair_sums = np.zeros(B, dtype=np.float64)
    for c in range(N_CORES):
        r = (np.asarray(res.results[c]["outd"]).astype(np.float64).sum(axis=0)
             + np.asarray(res.results[c]["outa"]).astype(np.float64).sum(axis=0))
        bce_sum += float(r[NACC])
        cols = r[:NACC].reshape(BLOC, NCH)
        chunk_a = a_pads[c].reshape(BLOC, NCH, 128).sum(axis=2)
        corr = np.where(dve_col.reshape(BLOC, NCH), KB * chunk_a, 0.0)
        pair_sums[c * BLOC:(c + 1) * BLOC] = (cols - corr).sum(axis=1)
        for rloc in overflows[c]:
            b = c * BLOC + rloc
            pos = v[b][y[b] == 1.0].astype(np.float64)
            neg = v[b][y[b] == 0.0].astype(np.float64) + MARGIN
            d = neg[None, :] - pos[:, None]
            pair_sums[b] = np.maximum(d, 0.0).sum()

    n_pos = y.sum(axis=1).astype(np.float64)
    n_neg = N - n_pos
    n_pairs = n_pos * n_neg
    valid = n_pairs > 0
    per_batch_mean = np.where(valid, pair_sums / np.maximum(n_pairs, 1.0), 0.0)
    valid_count = valid.sum()
    rank_loss = per_batch_mean.sum() / valid_count if valid_count > 0 else 0.0
    bce_loss = bce_sum / (B * N)
    return np.float32(bce_loss + rank_loss)


# revision 51
# speedup vs baseline: 1.1323x; 1.1323x over previous
"""Trainium2 Bass kernel for CombinedRankingLoss (BCE + pairwise margin ranking).

Full inputs: logits/labels/weights [64, 1024, 1] f32. Output: scalar f32.

Data-parallel over batch: 8 cores x 8 batches. The pairwise term
    T_b = sum_{i in pos} sum_{j in neg} relu((v_j + M) - v_i)
is computed per batch with the candidates PERMUTED on host (the loss is
invariant to per-batch candidate order):
  - a side (i): pos-compacted, padded to KA=640 with +BIG  (sent negated)
  - b side (j): neg-compacted (v+M), padded to KB=640 with -BIG, and sent
    as a bf16 hi/lo pair whose sum reconstructs f32 to ~1e-4
Masked pads contribute exactly 0 through the relu.

Per batch on device:
  - PE: one [16,128] bf16 selector matmul broadcasts (hi_b + lo_b) rows to a
    [128, KB] f32 PSUM tile.
  - 5 chunk ops (128 pos candidates each) fused compare+row-reduce, one
    instruction each, split across ScalarE and VectorE (both read PSUM):
      ScalarE: activation(Relu, bias=-a_col, accum_out)  -> sum_j relu(b_j-a_i)
      VectorE: tensor_scalar(op0=max a_col, op1=add-as-REDUCER, accum_out)
               -> sum_j max(b_j, a_i); host subtracts KB*a_i (exact identity
               sum_j max(b_j,a)-KB*a = sum_j relu(b_j-a))
  - BCE partials on flat [128,64] tiles (softplus = ln(1+exp), one ACT table
    set shared with Relu).
  - ones-matmul reduces accumulator partitions -> [1,48] -> DMA out.
Host: per-batch counts/means/valid handling + final scalar (f64).
"""
import sys
import numpy as np

sys.path.insert(0, "/opt/trn_rl_repo")

B, N = 64, 1024
N_CORES = 8
BLOC = B // N_CORES          # batches per core
KA = 512                     # fixed pos rows per batch; 4 chunks of 128
KB = 576                     # padded neg-count (j side, free dim); overflow
                             # falls back to exact host recompute per batch
NCH = KA // 128              # fixed chunks per batch (4)
NOVF = 2                     # shared overflow chunks (pos rows beyond 512,
                             # mixed batches; host attributes per partition)
MARGIN = 0.5
BIG = 16.0                   # mask sentinel; dominates |v|+margin (|v|<~5.5)
NACC = NCH * BLOC + NOVF     # 34 accumulator columns
NOUT = 48
SELW = N + NOVF * 128        # selector width: 8 batch blocks + overflow blocks

_CACHE = {}


def _patch_bass(bass):
    """Split multi-wait instructions (old walrus TPB_CTRL takes 1 wait)."""
    import json as _json
    if getattr(bass.Bass, "_wait_split_patched", False):
        return
    _orig = bass.Bass.to_json_bytes

    def _split(bir, limit=1):
        m = _json.loads(bir)
        for fn in m["functions"]:
            for bb in fn["blocks"]:
                out = []
                for i in bb.get("instructions", []):
                    si = i.get("sync_info") or {}
                    ow = si.get("on_wait") or []
                    if len(ow) > limit:
                        extra, keep = ow[:-limit], ow[-limit:]
                        for k, w in enumerate(extra):
                            out.append({
                                "debug": i.get("debug"), "engine": i["engine"],
                                "ins": [], "outs": [],
                                "name": i["name"] + f"_ws{k}",
                                "opcode": "NoOp",
                                "sync_info": {"on_wait": [w]},
                            })
                        si = dict(si)
                        si["on_wait"] = keep
                        i = dict(i)
                        i["sync_info"] = si
                    out.append(i)
                bb["instructions"] = out
        return _json.dumps(m).encode()

    bass.Bass.to_json_bytes = lambda self: _split(_orig(self))
    bass.Bass._wait_split_patched = True


def _engine_for(k):
    # 17:17 split over 32 fixed + 2 overflow chunk ops
    if k >= NCH * BLOC:                      # overflow chunks
        return "dve" if (k - NCH * BLOC) % 2 == 0 else "act"
    b, c = divmod(k, NCH)
    return "dve" if (c + b) % 2 == 0 else "act"


def _build(bass, tile, mybir):
    f32 = mybir.dt.float32
    bf16 = mybir.dt.bfloat16
    Alu = mybir.AluOpType
    Act = mybir.ActivationFunctionType

    nc = bass.Bass()
    na_d = nc.declare_dram_parameter("na", [BLOC, KA], f32, isOutput=False)
    b2_d = nc.declare_dram_parameter("b2", [16, KB], bf16, isOutput=False)
    v_d = nc.declare_dram_parameter("v", [128, 64], f32, isOutput=False)
    y_d = nc.declare_dram_parameter("y", [128, 64], f32, isOutput=False)
    w_d = nc.declare_dram_parameter("w", [128, 64], f32, isOutput=False)
    sel_d = nc.declare_dram_parameter("sel", [16, SELW], bf16, isOutput=False)
    naov_d = nc.declare_dram_parameter("naov", [128, 2 * NOVF], f32, isOutput=False)
    id8_d = nc.declare_dram_parameter("id8", [8, 8], f32, isOutput=False)
    outd_d = nc.declare_dram_parameter("outd", [128, NOUT], f32, isOutput=True)
    outa_d = nc.declare_dram_parameter("outa", [128, NOUT], f32, isOutput=True)

    with tile.TileContext(nc) as tc:
        with (
            tc.tile_pool(name="const", bufs=1) as const,
            tc.tile_pool(name="work", bufs=2) as work,
            tc.tile_pool(name="psum", bufs=3, space="PSUM") as psum,
            tc.tile_pool(name="psum1", bufs=1, space="PSUM") as psum1,
        ):
            # ---------- loads (matmul/transpose inputs first) ----------
            b2 = const.tile([16, KB], bf16)
            nc.sync.dma_start(out=b2[:], in_=b2_d[:])
            sel16 = const.tile([16, SELW], bf16)
            nc.sync.dma_start(out=sel16[:], in_=sel_d[:])
            na_rows = const.tile([BLOC, KA], f32)
            nc.sync.dma_start(out=na_rows[:], in_=na_d[:])
            # overflow a-columns, column layout direct from host: [-a | +a]
            naov = const.tile([128, 2 * NOVF], f32)
            nc.sync.dma_start(out=naov[:], in_=naov_d[:])
            ident8 = const.tile([8, 8], f32)
            nc.sync.dma_start(out=ident8[:], in_=id8_d[:])
            v_flat = const.tile([128, 64], f32)
            y_flat = const.tile([128, 64], f32)
            w_flat = const.tile([128, 64], f32)
            nc.sync.dma_start(out=v_flat[:], in_=v_d[:])
            nc.sync.dma_start(out=y_flat[:], in_=y_d[:])
            nc.sync.dma_start(out=w_flat[:], in_=w_d[:])

            # accumulators
            dve_acc = const.tile([128, NOUT], f32)
            act_acc = const.tile([128, NOUT], f32)
            nc.vector.memset(dve_acc[:], 0.0)
            nc.gpsimd.memset(act_acc[:], 0.0)

            # ---------- a-columns via PE transpose ----------
            tp = psum1.tile([128, NCH * 8], f32)
            for c in range(NCH):
                nc.tensor.transpose(tp[:, c * 8:(c + 1) * 8],
                                    na_rows[:, c * 128:(c + 1) * 128], ident8[:])
            na_cols = const.tile([128, NCH * 8], f32)   # -a  (ACT bias)
            nc.vector.tensor_copy(out=na_cols[:], in_=tp[:])
            a_cols = const.tile([128, NCH * 8], f32)    # +a  (DVE max operand)
            nc.vector.tensor_scalar(out=a_cols[:], in0=tp[:], scalar1=-1.0,
                                    scalar2=None, op0=Alu.mult)

            # ---------- BCE (flat [128,64]) ----------
            sp = work.tile([128, 64], f32)
            nc.scalar.activation(out=sp[:], in_=v_flat[:], func=Act.Exp)
            nc.vector.tensor_scalar(out=sp[:], in0=sp[:], scalar1=1.0,
                                    scalar2=None, op0=Alu.add)
            nc.scalar.activation(out=sp[:], in_=sp[:], func=Act.Ln)
            xy = work.tile([128, 64], f32)
            nc.vector.tensor_tensor(out=xy[:], in0=v_flat[:], in1=y_flat[:], op=Alu.mult)
            nc.vector.tensor_tensor(out=xy[:], in0=sp[:], in1=xy[:], op=Alu.subtract)
            bce_scr = work.tile([128, 64], f32)
            nc.vector.scalar_tensor_tensor(
                out=bce_scr[:], in0=xy[:], scalar=1.0, op0=Alu.mult,
                op1=Alu.mult, in1=w_flat[:], accum_out=dve_acc[:, NACC:NACC + 1])

            # ---------- main pairwise loop (software-pipelined emission) ----
            def emit_mm(b):
                bcp = psum.tile([128, KB], f32, tag="bcp")    # pool pads to banks
                lhsT = sel16[:, b * 128:(b + 1) * 128]
                nc.tensor.matmul(bcp[:, 0:512], lhsT, b2[:, 0:512],
                                 start=True, stop=True)
                nc.tensor.matmul(bcp[:, 512:KB], lhsT, b2[:, 512:KB],
                                 start=True, stop=True)
                return bcp

            def emit_chunk(k, bcp, nac, ac):
                col = k
                if _engine_for(k) == "act":
                    scr_act = work.tile([128, KB], f32, tag="scr_act")
                    nc.scalar.activation(
                        out=scr_act[:], in_=bcp[:], func=Act.Relu,
                        bias=nac, scale=1.0,
                        accum_out=act_acc[:, col:col + 1])
                else:
                    scr_dve = work.tile([128, KB], f32, tag="scr_dve")
                    nc.vector.tensor_scalar(
                        out=scr_dve[:], in0=bcp[:], scalar1=ac, scalar2=None,
                        op0=Alu.max, op1=Alu.add,
                        accum_out=dve_acc[:, col:col + 1])

            # block index sequence: 8 batch blocks then NOVF overflow blocks;
            # selector column block i covers sel16[:, i*128:(i+1)*128]
            bcp_q = [emit_mm(b) for b in range(3)]
            for b in range(BLOC + NOVF):
                bcp = bcp_q[b % 3]
                if b < BLOC:
                    ids = list(range(NCH))
                    ids.sort(key=lambda c: _engine_for(b * NCH + c) != "dve")
                    for c in ids:
                        k = b * NCH + c
                        cidx = c * 8 + b
                        emit_chunk(k, bcp, na_cols[:, cidx:cidx + 1],
                                   a_cols[:, cidx:cidx + 1])
                else:
                    j = b - BLOC
                    k = NCH * BLOC + j
                    emit_chunk(k, bcp, naov[:, j:j + 1],
                               naov[:, NOVF + j:NOVF + j + 1])
                if b + 3 < BLOC + NOVF:
                    bcp_q[(b + 3) % 3] = emit_mm(b + 3)

            # ---------- results out (host does the partition reduction) ----
            nc.sync.dma_start(out=outd_d[:], in_=dve_acc[:])
            nc.sync.dma_start(out=outa_d[:], in_=act_acc[:])

    return nc


def _get_nc():
    if "nc" not in _CACHE:
        import concourse.bass as bass
        import concourse.tile as tile
        from concourse import mybir
        _patch_bass(bass)
        _CACHE["nc"] = _build(bass, tile, mybir)
    return _CACHE["nc"]


def _prep_core(v, y):
    """Compact one core's batches. Returns na [BLOC,KA] f32 (negated a, pads
    -BIG), b2 [16,KB] bf16 (hi/lo), naov [128,2*NOVF] f32 ([-a | +a] overflow
    columns), ovf_batch [NOVF*128] int (partition -> local batch, -1 pad),
    ovf_a [NOVF*128] f64, a_pad [BLOC,KA] f64, overflow list (host fallback)."""
    import ml_dtypes
    na = np.full((BLOC, KA), -BIG, dtype=np.float32)
    b_pad = np.full((BLOC, KB), -BIG, dtype=np.float32)
    ovf_batch = np.full(NOVF * 128, -1, dtype=np.int64)
    ovf_a = np.full(NOVF * 128, BIG, dtype=np.float64)
    overflow = []
    ptr = 0
    for r in range(BLOC):
        pos = v[r][y[r] == 1.0]
        neg = v[r][y[r] == 0.0] + np.float32(MARGIN)
        extra = len(pos) - KA
        if len(neg) > KB or (extra > 0 and ptr + extra > NOVF * 128):
            overflow.append(r)   # leave na row at -BIG => device contributes 0
            continue
        npos = min(len(pos), KA)
        na[r, :npos] = -pos[:npos]
        if extra > 0:
            ovf_a[ptr:ptr + extra] = pos[KA:].astype(np.float64)
            ovf_batch[ptr:ptr + extra] = r
            ptr += extra
        b_pad[r, :len(neg)] = neg
    naov = np.zeros((128, 2 * NOVF), dtype=np.float32)
    for j in range(NOVF):
        seg = ovf_a[j * 128:(j + 1) * 128].astype(np.float32)
        naov[:, j] = -seg
        naov[:, NOVF + j] = seg
    hi = b_pad.astype(ml_dtypes.bfloat16)
    lo = (b_pad - hi.astype(np.float32)).astype(ml_dtypes.bfloat16)
    b2 = np.concatenate([hi, lo], axis=0)           # [16, KB]
    return na, b2, naov, ovf_batch, ovf_a, -na.astype(np.float64), overflow


def make_in_maps(v, y, w):
    import ml_dtypes
    id8 = np.eye(8, dtype=np.float32)
    in_maps, a_pads, overflows, ovf_batches, ovf_as = [], [], [], [], []
    for c in range(N_CORES):
        sl = slice(c * BLOC, (c + 1) * BLOC)
        na, b2, naov, ovf_batch, ovf_a, a_pad, ovf = _prep_core(v[sl], y[sl])
        a_pads.append(a_pad)
        overflows.append(ovf)
        ovf_batches.append(ovf_batch)
        ovf_as.append(ovf_a)
        sel = np.zeros((16, SELW), dtype=np.float32)
        for b in range(BLOC):
            sel[b, b * 128:(b + 1) * 128] = 1.0
            sel[b + 8, b * 128:(b + 1) * 128] = 1.0
        for j in range(NOVF):
            for p in range(128):
                bb = ovf_batch[j * 128 + p]
                if bb >= 0:
                    sel[bb, N + j * 128 + p] = 1.0
                    sel[bb + 8, N + j * 128 + p] = 1.0
        in_maps.append({
            "na": na, "b2": b2, "naov": naov,
            "v": v[sl].reshape(128, 64),
            "y": y[sl].reshape(128, 64),
            "w": w[sl].reshape(128, 64),
            "sel": sel.astype(ml_dtypes.bfloat16), "id8": id8,
        })
    return in_maps, a_pads, overflows, ovf_batches, ovf_as


def kernel(logits, labels, weights):
    from concourse.bass_utils import run_bass_kernel_spmd

    nc = _get_nc()
    v = np.ascontiguousarray(logits.reshape(B, N), dtype=np.float32)
    y = np.ascontiguousarray(labels.reshape(B, N), dtype=np.float32)
    w = np.ascontiguousarray(weights.reshape(B, N), dtype=np.float32)

    in_maps, a_pads, overflows, ovf_batches, ovf_as = make_in_maps(v, y, w)
    res = run_bass_kernel_spmd(nc, in_maps, list(range(N_CORES)))

    # ---------- host combine ----------
    NB = NCH * BLOC
    dve_col = np.array([_engine_for(k) == "dve" for k in range(NB)])
    bce_sum = 0.0
    pair_sums = np.zeros(B, dtype=np.float64)
    for c in range(N_CORES):
        per_p = (np.asarray(res.results[c]["outd"]).astype(np.float64)
                 + np.asarray(res.results[c]["outa"]).astype(np.float64))
        r = per_p.sum(axis=0)
        bce_sum += float(r[NACC])
        cols = r[:NB].reshape(BLOC, NCH)
        chunk_a = a_pads[c].reshape(BLOC, NCH, 128).sum(axis=2)
        corr = np.where(dve_col.reshape(BLOC, NCH), KB * chunk_a, 0.0)
        pair_sums[c * BLOC:(c + 1) * BLOC] = (cols - corr).sum(axis=1)
        for j in range(NOVF):
            col = NB + j
            vals = per_p[:, col].copy()
            if _engine_for(col) == "dve":
                vals -= KB * ovf_as[c][j * 128:(j + 1) * 128]
            for p in range(128):
                bb = ovf_batches[c][j * 128 + p]
                if bb >= 0:
                    pair_sums[c * BLOC + bb] += vals[p]
        for rloc in overflows[c]:
            b = c * BLOC + rloc
            pos = v[b][y[b] == 1.0].astype(np.float64)
            neg = v[b][y[b] == 0.0].astype(np.float64) + MARGIN
            d = neg[None, :] - pos[:, None]
            pair_sums[b] = np.maximum(d, 0.0).sum()

    n_pos = y.sum(axis=1).astype(np.float64)
    n_neg = N - n_pos
    n_pairs = n_pos * n_neg
    valid = n_pairs > 0
    per_batch_mean = np.where(valid, pair_sums / np.maximum(n_pairs, 1.0), 0.0)
    valid_count = valid.sum()
    rank_loss = per_batch_mean.sum() / valid_count if valid_count > 0 else 0.0
    bce_loss = bce_sum / (B * N)
    return np.float32(bce_loss + rank_loss)
